# revision 21
# baseline (speedup 1.0000x reference)
"""Trainium2 Bass kernel for the low-rank linear operator.

Math: the reference collapses algebraically. With y = linspace(-1,1,H),
x = linspace(-1,1,W), dx = 2/(W-1):

  Vy[b,i] = sum_{h,w} v[b,i,h,w] * y_h
  Vx[b,i] = sum_{h,w} v[b,i,h,w] * x_w
  inner[b,r] = dx * sum_i (Vy[b,i]*psi[r,i,0] + Vx[b,i]*psi[r,i,1])
  A[b,o] = sum_r inner[b,r]*phi[o,r,0];  Bc[b,o] = sum_r inner[b,r]*phi[o,r,1]
  u[b,o,h,w] = A[b,o]*y_h + Bc[b,o]*x_w

Sharding: data-parallel over batch, 2 batches per core, 8 cores, no
collectives.

The problem is HBM-bandwidth bound on the way in and generation-engine
bound on the way out. Input streams as bf16 (halves read traffic; rel
err vs the 2e-2 gate measures ~3e-3). Output is written as int8 with a
per-(b,o) dequant scale s = (|A|+|B|)/127 (max|u| = |A|+|B|, so no
clipping); the host multiplies back. int8 quarters write traffic vs f32.

Input reduction: host pre-transposes v to [b, p=h//2, i, hh, w] bf16
(16KB-contiguous per-partition DMA descriptors). For each channel ch one
matmul with a sliding-window lhsT (zeros except col 2ch -> y_even, col
2ch+1 -> ones) over rhs [128, (hh,w)=512] accumulates, for ALL 64
channels, y_even-weighted row sums (psum row 2ch) and column sums (row
2ch+1) into one [128, 512] f32 psum bank. Full-width DVE mult+reduce
passes against wty/wtx give gy/gx; tiny f32 matmuls (contraction-side
lhsT, no transposes) produce inner -> (A,B) -> quantized A' = 127*A/s,
B' = 127*B/s.

Output generation uses an o-on-partitions layout u6[b, p=2o+j, h', w]
(h = j*128 + h') so per-o scalars become per-partition: two tiny PE
outer products build yA[p,h'] = A'[o]*y[j*128+h'] and xB[p,w] =
B'[o]*x[w], then u = yA + xB is generated by WIDE broadcast
tensor_tensor ops (DVE + Pool, ~7K elems/lane per instruction) plus
per-row ACT activations (bias=yA column) -- ~3x fewer engine-cycles of
overhead than per-(o,hh) tensor_scalar tiles. The host reassembly is a
pure reshape (p splits as (o,j) in order) times the dequant scale.

The tiny chain and gen units of batch 0 are emitted interleaved into
batch 1's chunk loop so the PE queue never stalls and the generation
engines stay fed while batch 1 streams in.
"""

import sys

try:
    import concourse.bass as bass  # noqa: F401
except ImportError:
    for _p in ("/opt/trn_rl_repo", "/root/.axon_site/_ro/trn_rl_repo"):
        if _p not in sys.path:
            sys.path.insert(0, _p)

import numpy as np

import concourse.bacc as bacc
import concourse.bass as bass
import concourse.mybir as mybir
import concourse.tile as tile
from concourse.bass import broadcast_tensor_aps
from concourse.bass_utils import run_bass_kernel_spmd

F32 = mybir.dt.float32
BF16 = mybir.dt.bfloat16
I8 = mybir.dt.int8
MULT = mybir.AluOpType.mult
ADD = mybir.AluOpType.add
MAX = mybir.AluOpType.max

B, CI, CO, R, H, W = 16, 64, 64, 64, 256, 256
N_CORES = 8
BPC = B // N_CORES  # batches per core
HP = H // 2         # h-pairs per partition dim
HQ = H // 2         # h' extent in the output layout (h = j*128 + h')

IBLK = 8            # input channels per DMA (1MB bf16, 8KB descriptors);
                    # small chunks keep the PE busy-fraction high (HAM warm)
NIB = CI // IBLK

# gen units: (engine, h'0, h'1) -- DVE gets wide broadcast-TT spans (f32
# in -> int8 out TT is DVE-only); ACT gets activation rows (bias=yA col);
# Pool gets tensor_scalar rows. Shares follow measured engine rates.
_GEN_UNITS = (
    ("dve", 0, 22),
    ("pool", 64, 78),
    ("act", 92, 104),
    ("dve", 22, 44),
    ("pool", 78, 92),
    ("act", 104, 116),
    ("dve", 44, 64),
    ("act", 116, 128),
)

# packed-constant column offsets (cf32 [128, CF32_W] f32)
_WTY = 0            # [128, 512]
_WTX = 512          # [128, 512]; row 1 cols [512:768) is also x (rhs for xB)
_PSIY = 1024        # [128, 64]
_PSIX = 1088        # [128, 64]
_PHI = 1152         # [64, 128]
_YSEL = 1280        # [128, 128]: Ysel[p, h'] = y[(p%2)*128 + h']
_XREP2 = 1408       # [128, 256]: x replicated on all partitions
_ID1 = 1664         # [1, 1]
CF32_W = 1665
# cf16 [128, CBF16_W] bf16: sliding-window lhsT table
_YLHS = 0           # [128, 384]: col 128 = y_even, col 129 = ones
CBF16_W = 384


def build_nc():
    nc = bacc.Bacc("TRN2", target_bir_lowering=False, debug=False)

    v5 = nc.dram_tensor("v5", [BPC, HP, CI, 2, W], BF16, kind="ExternalInput")
    cf32d = nc.dram_tensor("cf32", [128, CF32_W], F32, kind="ExternalInput")
    cf16d = nc.dram_tensor("cf16", [128, CBF16_W], BF16, kind="ExternalInput")
    u6 = nc.dram_tensor("u6", [BPC, 128, HQ, W], I8, kind="ExternalOutput")
    sdeq = nc.dram_tensor("sdeq", [BPC, CO], F32, kind="ExternalOutput")

    with tile.TileContext(nc) as tc:
        with (
            tc.tile_pool(name="consts", bufs=1) as consts,
            tc.tile_pool(name="inp", bufs=4) as in_pool,
            tc.tile_pool(name="outp", bufs=5) as out_pool,
            tc.tile_pool(name="scr", bufs=3) as scratch,
            tc.tile_pool(name="gen", bufs=4) as gen_pool,
            tc.tile_pool(name="psumA", bufs=2, space="PSUM") as psum_a,
            tc.tile_pool(name="psumT", bufs=1, space="PSUM") as psum_t,
            tc.tile_pool(name="psumG", bufs=2, space="PSUM") as psum_g,
        ):
            # cf16 gates the first matmul: tiny, lands first on the sync ring
            # ahead of the v reads; cf32 rides the scalar ring.
            cf16 = consts.tile([128, CBF16_W], BF16)
            nc.sync.dma_start(cf16[:], cf16d[:])
            cf32 = consts.tile([128, CF32_W], F32)
            nc.scalar.dma_start(cf32[:], cf32d[:])

            wty = cf32[:, _WTY : _WTY + 2 * W]
            wtx = cf32[:, _WTX : _WTX + 2 * W]
            psi2y = cf32[:, _PSIY : _PSIY + R]
            psi2x = cf32[:, _PSIX : _PSIX + R]
            phicat = cf32[0:R, _PHI : _PHI + 2 * CO]
            ysel = cf32[:, _YSEL : _YSEL + 128]
            xrep2 = cf32[:, _XREP2 : _XREP2 + W]
            id1 = cf32[0:1, _ID1 : _ID1 + 1]

            # per-batch reduction vectors, one column per batch
            gy_sb = consts.tile([2 * CI, BPC], F32)
            gx_sb = consts.tile([2 * CI, BPC], F32)

            def stage_a(b, interleave=None):
                """Reduce v[b] -> gy_sb/gx_sb[:, b]."""
                inter = interleave() if interleave is not None else None
                ps = psum_a.tile([128, 2, W], F32, tag="A")
                for blk in range(NIB):
                    t = in_pool.tile([128, IBLK, 2, W], BF16, tag="in")
                    nc.sync.dma_start(
                        t[:],
                        v5[b, :, blk * IBLK : (blk + 1) * IBLK, :, :],
                    )
                    for ii in range(IBLK):
                        ch = blk * IBLK + ii
                        lo = _YLHS + 128 - 2 * ch
                        nc.tensor.matmul(
                            ps[:],
                            lhsT=cf16[:, lo : lo + 128],
                            rhs=t[:, ii, :, :],
                            start=(ch == 0),
                            stop=(ch == CI - 1),
                        )
                    if inter is not None:
                        next(inter, None)
                s2 = scratch.tile([128, 2 * W], F32, tag="s2")
                nc.vector.tensor_copy(s2[:], ps[:].rearrange("p hh w -> p (hh w)"))
                sc = scratch.tile([128, 2 * W], F32, tag="sc")
                nc.vector.tensor_tensor(out=sc[:], in0=s2[:], in1=wty, op=MULT)
                nc.vector.tensor_reduce(
                    out=gy_sb[:, b : b + 1], in_=sc[:],
                    axis=mybir.AxisListType.X, op=ADD,
                )
                sc2 = scratch.tile([128, 2 * W], F32, tag="sc")
                nc.vector.tensor_tensor(out=sc2[:], in0=s2[:], in1=wtx, op=MULT)
                nc.vector.tensor_reduce(
                    out=gx_sb[:, b : b + 1], in_=sc2[:],
                    axis=mybir.AxisListType.X, op=ADD,
                )

            def tiny_gen(b, out):
                """gy/gx[:, b] -> yA/xB gen bases (emitted in 3 pumps)."""
                # inner, transposed directly via contraction-side lhsT
                innert_ps = psum_t.tile([R, 1], F32, tag="tiny")
                nc.tensor.matmul(
                    innert_ps[:], lhsT=psi2y, rhs=gy_sb[:, b : b + 1],
                    start=True, stop=False,
                )
                nc.tensor.matmul(
                    innert_ps[:], lhsT=psi2x, rhs=gx_sb[:, b : b + 1],
                    start=False, stop=True,
                )
                sb_innert = scratch.tile([R, 1], F32, tag="ti2")
                nc.vector.tensor_copy(sb_innert[:], innert_ps[:])
                yield

                ab_ps = psum_t.tile([1, 2 * CO], F32, tag="tiny")
                nc.tensor.matmul(
                    ab_ps[:], lhsT=sb_innert[:], rhs=phicat,
                    start=True, stop=True,
                )
                sb_ab = scratch.tile([1, 2 * CO], F32, tag="ti3")
                nc.vector.tensor_copy(sb_ab[:], ab_ps[:])

                # s_raw[o] = |A[o]| + |B[o]|; ab2 = 127 * ab / s_raw;
                # sdeq[b, o] = s_raw / 127 for host dequantization
                negab = scratch.tile([1, 2 * CO], F32, tag="ti4b")
                nc.vector.tensor_scalar(
                    out=negab[:], in0=sb_ab[:], scalar1=-1.0,
                    scalar2=None, op0=MULT,
                )
                absab = scratch.tile([1, 2 * CO], F32, tag="ti4")
                nc.vector.tensor_tensor(
                    out=absab[:], in0=sb_ab[:], in1=negab[:], op=MAX
                )
                a2 = absab[0:1, :].rearrange("r (o c) -> r o c", c=2)
                sraw = scratch.tile([1, CO], F32, tag="ti5")
                nc.vector.tensor_tensor(
                    out=sraw[:], in0=a2[:, :, 0], in1=a2[:, :, 1], op=ADD
                )
                rcp = scratch.tile([1, CO], F32, tag="ti6b")
                nc.vector.reciprocal(rcp[:], sraw[:])
                rinv = scratch.tile([1, CO], F32, tag="ti6")
                nc.vector.tensor_scalar(
                    out=rinv[:], in0=rcp[:], scalar1=127.0,
                    scalar2=None, op0=MULT,
                )
                ab2 = scratch.tile([1, 2 * CO], F32, tag="ti7")
                ab2v = ab2[0:1, :].rearrange("r (o c) -> r o c", c=2)
                abv = sb_ab[0:1, :].rearrange("r (o c) -> r o c", c=2)
                nc.vector.tensor_tensor(
                    out=ab2v[:, :, 0], in0=abv[:, :, 0], in1=rinv[:], op=MULT
                )
                nc.vector.tensor_tensor(
                    out=ab2v[:, :, 1], in0=abv[:, :, 1], in1=rinv[:], op=MULT
                )
                sdeq_sb = scratch.tile([1, CO], F32, tag="ti8")
                nc.vector.tensor_scalar(
                    out=sdeq_sb[:], in0=sraw[:], scalar1=1.0 / 127.0,
                    scalar2=None, op0=MULT,
                )
                nc.scalar.dma_start(sdeq[b : b + 1, :], sdeq_sb[:])
                yield

                # rows with A'/B' duplicated per (o, j) pair, then PE
                # transposes to per-partition scalar columns
                arow = scratch.tile([1, 128], F32, tag="ti9")
                arv = arow[0:1, :].rearrange("r (o j) -> r o j", j=2)
                nc.vector.tensor_copy(arv[:, :, 0], ab2v[:, :, 0])
                nc.vector.tensor_copy(arv[:, :, 1], ab2v[:, :, 0])
                brow = scratch.tile([1, 128], F32, tag="ti10")
                brv = brow[0:1, :].rearrange("r (o j) -> r o j", j=2)
                nc.vector.tensor_copy(brv[:, :, 0], ab2v[:, :, 1])
                nc.vector.tensor_copy(brv[:, :, 1], ab2v[:, :, 1])
                acol_ps = psum_g.tile([128, 1], F32, tag="ac")
                nc.tensor.transpose(acol_ps[:], arow[:], id1)
                acol = scratch.tile([128, 1], F32, tag="ti11")
                nc.vector.tensor_copy(acol[:], acol_ps[:])
                bcol_ps = psum_g.tile([128, 1], F32, tag="bc")
                nc.tensor.transpose(bcol_ps[:], brow[:], id1)
                bcol = scratch.tile([128, 1], F32, tag="ti12")
                nc.vector.tensor_copy(bcol[:], bcol_ps[:])

                # yA[p, h'] = A'[o] * y[j*128+h'];  xB[p, w] = B'[o] * x[w]
                ya = gen_pool.tile([128, 128], F32, tag="yat")
                nc.vector.tensor_scalar(
                    out=ya[:], in0=ysel, scalar1=acol[:],
                    scalar2=None, op0=MULT,
                )
                xb = gen_pool.tile([128, W], F32, tag="xbt")
                nc.vector.tensor_scalar(
                    out=xb[:], in0=xrep2, scalar1=bcol[:],
                    scalar2=None, op0=MULT,
                )
                out["ya"], out["xb"] = ya, xb
                yield

            def stage_c_gen(b, bases):
                for eng, h0, h1 in _GEN_UNITS:
                    yield
                    n = h1 - h0
                    ya, xb = bases["ya"], bases["xb"]
                    ot = out_pool.tile([128, n, W], I8, tag=f"o{n}")
                    if eng == "act":
                        for hh in range(h0, h1):
                            nc.scalar.activation(
                                ot[:, hh - h0, :], xb[:],
                                mybir.ActivationFunctionType.Identity,
                                bias=ya[:, hh : hh + 1],
                            )
                    elif eng == "pool":
                        for hh in range(h0, h1):
                            nc.gpsimd.tensor_scalar(
                                out=ot[:, hh - h0, :], in0=xb[:],
                                scalar1=1.0, scalar2=ya[:, hh : hh + 1],
                                op0=MULT, op1=ADD,
                            )
                    else:
                        in0 = ya[:, h0:h1].rearrange(
                            "p (h one) -> p h one", one=1
                        )
                        in1 = xb[:].rearrange("p (one w) -> p one w", one=1)
                        i0, i1 = broadcast_tensor_aps(in0, in1)
                        e = nc.vector if eng == "dve" else nc.gpsimd
                        e.tensor_tensor(out=ot[:], in0=i0, in1=i1, op=ADD)
                    nc.scalar.dma_start(u6[b, :, h0:h1, :], ot[:])

            stage_a(0)

            state = {}

            def inter0():
                # Pumped once per A1 chunk: the tiny chain lands in 3 pieces
                # (each PE hop's DVE-side input was produced during the
                # previous chunk's matmuls, so the PE queue never stalls),
                # then gen-0 units flow while batch 1 streams in.
                bases = {}
                state["bases"] = bases
                tg = tiny_gen(0, bases)
                yield from tg
                state["g"] = stage_c_gen(0, bases)
                next(state["g"], None)
                next(state["g"], None)  # unit 0
                yield
                next(state["g"], None)  # unit 1
                yield

            stage_a(1, interleave=inter0)
            bases1 = {}
            for _ in tiny_gen(1, bases1):
                pass
            for _ in state["g"]:        # gen-0 units 2-5
                pass
            for _ in stage_c_gen(1, bases1):
                pass

    nc.compile()
    return nc


def make_in_maps(v, psi, phi):
    import ml_dtypes
    bf16 = ml_dtypes.bfloat16
    y = np.linspace(-1.0, 1.0, H, dtype=np.float32)
    x = np.linspace(-1.0, 1.0, W, dtype=np.float32)
    dx = np.float32(2.0 / (W - 1))
    dy = np.float32(2.0 / (H - 1))

    cf32 = np.zeros((128, CF32_W), np.float32)
    # wty: row 2i = 1 (y_even-weighted sums); row 2i+1 cols [W:2W) = dy
    cf32[0::2, _WTY : _WTY + 2 * W] = 1.0
    cf32[1::2, _WTY + W : _WTY + 2 * W] = dy
    # wtx: row 2i+1 = x (both hh halves); row 1 cols [512:768) doubles as
    # the xB outer-product rhs
    cf32[1::2, _WTX : _WTX + W] = x
    cf32[1::2, _WTX + W : _WTX + 2 * W] = x
    # psi packs (dx folded in)
    cf32[0::2, _PSIY : _PSIY + R] = psi[:, :, 0].T * dx
    cf32[1::2, _PSIY : _PSIY + R] = psi[:, :, 0].T * dx
    cf32[1::2, _PSIX : _PSIX + R] = psi[:, :, 1].T * dx
    # phicat[r, 2o+c] = phi[o, r, c]
    cf32[0:R, _PHI : _PHI + 2 * CO] = np.stack(
        [phi[:, :, 0].T, phi[:, :, 1].T], axis=2
    ).reshape(R, 2 * CO)
    cf32[0::2, _YSEL : _YSEL + 128] = y[:128]
    cf32[1::2, _YSEL : _YSEL + 128] = y[128:]
    cf32[:, _XREP2 : _XREP2 + W] = x
    cf32[0, _ID1] = 1.0

    cf16 = np.zeros((128, CBF16_W), np.float32)
    cf16[:, _YLHS + 128] = y[0::2]
    cf16[:, _YLHS + 129] = 1.0
    cf16 = cf16.astype(bf16)

    # v[b, i, h, w] -> [b, p, i, hh, w] bf16
    v16 = v.astype(bf16)
    vt = v16.reshape(B, CI, HP, 2, W).transpose(0, 2, 1, 3, 4)

    common = {"cf32": cf32, "cf16": cf16}
    return [
        {
            "v5": np.ascontiguousarray(vt[BPC * c : BPC * (c + 1)]),
            **common,
        }
        for c in range(N_CORES)
    ]


def gather_out(results):
    """u6 [BPC, 2o+j, h', w] int8 + sdeq [BPC, CO] -> u [B, CO, H, W] f32."""
    arr = np.stack([r["u6"] for r in results])   # [8, BPC, 128, HQ, W]
    sd = np.stack([r["sdeq"] for r in results])  # [8, BPC, CO]
    arr = arr.reshape(N_CORES, BPC, CO, 2, HQ, W)  # p -> (o, j), in order
    u = arr.astype(np.float32) * sd[:, :, :, None, None, None]
    return np.ascontiguousarray(u.reshape(B, CO, H, W))


_NC_CACHE = None


def kernel(v, psi, phi):
    global _NC_CACHE
    if _NC_CACHE is None:
        _NC_CACHE = build_nc()
    nc = _NC_CACHE
    in_maps = make_in_maps(
        np.asarray(v, dtype=np.float32),
        np.asarray(psi, dtype=np.float32),
        np.asarray(phi, dtype=np.float32),
    )
    res = run_bass_kernel_spmd(nc, in_maps, core_ids=list(range(N_CORES)))
    return gather_out(res.results)


if __name__ == "__main__":
    build_nc()
    print("build ok")


# revision 22
# speedup vs baseline: 1.0270x; 1.0270x over previous
"""Trainium2 Bass kernel for the low-rank linear operator.

Math: the reference collapses algebraically. With y = linspace(-1,1,H),
x = linspace(-1,1,W), dx = 2/(W-1):

  Vy[b,i] = sum_{h,w} v[b,i,h,w] * y_h
  Vx[b,i] = sum_{h,w} v[b,i,h,w] * x_w
  inner[b,r] = dx * sum_i (Vy[b,i]*psi[r,i,0] + Vx[b,i]*psi[r,i,1])
  A[b,o] = sum_r inner[b,r]*phi[o,r,0];  Bc[b,o] = sum_r inner[b,r]*phi[o,r,1]
  u[b,o,h,w] = A[b,o]*y_h + Bc[b,o]*x_w

Sharding: data-parallel over batch, 2 batches per core, 8 cores, no
collectives.

The problem is HBM-bandwidth bound on the way in and generation-engine
bound on the way out. Input streams as bf16 (halves read traffic; rel
err vs the 2e-2 gate measures ~3e-3). Output is written as int8 with a
per-(b,o) dequant scale s = (|A|+|B|)/127 (max|u| = |A|+|B|, so no
clipping); the host multiplies back. int8 quarters write traffic vs f32.

Input reduction: host pre-transposes v to [b, p=h//2, i, hh, w] bf16
(16KB-contiguous per-partition DMA descriptors). For each channel ch one
matmul with a sliding-window lhsT (zeros except col 2ch -> y_even, col
2ch+1 -> ones) over rhs [128, (hh,w)=512] accumulates, for ALL 64
channels, y_even-weighted row sums (psum row 2ch) and column sums (row
2ch+1) into one [128, 512] f32 psum bank. Full-width DVE mult+reduce
passes against wty/wtx give gy/gx; tiny f32 matmuls (contraction-side
lhsT, no transposes) produce inner -> (A,B) -> quantized A' = 127*A/s,
B' = 127*B/s.

Output generation uses an o-on-partitions layout u6[b, p=2o+j, h', w]
(h = j*128 + h') so per-o scalars become per-partition: two tiny PE
outer products build yA[p,h'] = A'[o]*y[j*128+h'] and xB[p,w] =
B'[o]*x[w], then u = yA + xB is generated by WIDE broadcast
tensor_tensor ops (DVE + Pool, ~7K elems/lane per instruction) plus
per-row ACT activations (bias=yA column) -- ~3x fewer engine-cycles of
overhead than per-(o,hh) tensor_scalar tiles. The host reassembly is a
pure reshape (p splits as (o,j) in order) times the dequant scale.

The tiny chain and gen units of batch 0 are emitted interleaved into
batch 1's chunk loop so the PE queue never stalls and the generation
engines stay fed while batch 1 streams in.
"""

import sys

try:
    import concourse.bass as bass  # noqa: F401
except ImportError:
    for _p in ("/opt/trn_rl_repo", "/root/.axon_site/_ro/trn_rl_repo"):
        if _p not in sys.path:
            sys.path.insert(0, _p)

import numpy as np

import concourse.bacc as bacc
import concourse.bass as bass
import concourse.mybir as mybir
import concourse.tile as tile
from concourse.bass import broadcast_tensor_aps
from concourse.bass_utils import run_bass_kernel_spmd

F32 = mybir.dt.float32
BF16 = mybir.dt.bfloat16
I8 = mybir.dt.int8
MULT = mybir.AluOpType.mult
ADD = mybir.AluOpType.add
MAX = mybir.AluOpType.max

B, CI, CO, R, H, W = 16, 64, 64, 64, 256, 256
N_CORES = 8
BPC = B // N_CORES  # batches per core
HP = H // 2         # h-pairs per partition dim
HQ = H // 2         # h' extent in the output layout (h = j*128 + h')

IBLK = 16           # input channels per DMA (2MB bf16, 16KB descriptors)
NIB = CI // IBLK

# gen units: (engine, h'0, h'1) -- DVE gets wide broadcast-TT spans (f32
# in -> int8 out TT is DVE-only); ACT gets activation rows (bias=yA col);
# Pool gets tensor_scalar rows. Shares follow measured engine rates.
_GEN_UNITS = (
    ("dve", 0, 25),
    ("act", 100, 111),
    ("dve", 25, 50),
    ("pool", 121, 128),
    ("dve", 50, 75),
    ("act", 111, 121),
    ("dve", 75, 100),
)

# packed-constant column offsets (cf32 [128, CF32_W] f32)
_WTY = 0            # [128, 512]
_WTX = 512          # [128, 512]; row 1 cols [512:768) is also x (rhs for xB)
_PSIY = 1024        # [128, 64]
_PSIX = 1088        # [128, 64]
_PHI = 1152         # [64, 128]
_YSEL = 1280        # [128, 128]: Ysel[p, h'] = y[(p%2)*128 + h']
_XREP2 = 1408       # [128, 256]: x replicated on all partitions
_ID1 = 1664         # [1, 1]
CF32_W = 1665
# cf16 [128, CBF16_W] bf16: sliding-window lhsT table
_YLHS = 0           # [128, 384]: col 128 = y_even, col 129 = ones
CBF16_W = 384


def build_nc():
    nc = bacc.Bacc("TRN2", target_bir_lowering=False, debug=False)

    v5 = nc.dram_tensor("v5", [BPC, HP, CI, 2, W], BF16, kind="ExternalInput")
    cf32d = nc.dram_tensor("cf32", [128, CF32_W], F32, kind="ExternalInput")
    cf16d = nc.dram_tensor("cf16", [128, CBF16_W], BF16, kind="ExternalInput")
    u6 = nc.dram_tensor("u6", [BPC, 128, HQ, W], I8, kind="ExternalOutput")
    sdeq = nc.dram_tensor("sdeq", [BPC, CO], F32, kind="ExternalOutput")

    with tile.TileContext(nc) as tc:
        with (
            tc.tile_pool(name="consts", bufs=1) as consts,
            tc.tile_pool(name="inp", bufs=4) as in_pool,
            tc.tile_pool(name="outp", bufs=5) as out_pool,
            tc.tile_pool(name="scr", bufs=3) as scratch,
            tc.tile_pool(name="gen", bufs=4) as gen_pool,
            tc.tile_pool(name="psumA", bufs=2, space="PSUM") as psum_a,
            tc.tile_pool(name="psumT", bufs=1, space="PSUM") as psum_t,
            tc.tile_pool(name="psumG", bufs=2, space="PSUM") as psum_g,
        ):
            # cf16 gates the first matmul: tiny, lands first on the sync ring
            # ahead of the v reads; cf32 rides the scalar ring.
            cf16 = consts.tile([128, CBF16_W], BF16)
            nc.sync.dma_start(cf16[:], cf16d[:])
            cf32 = consts.tile([128, CF32_W], F32)
            nc.scalar.dma_start(cf32[:], cf32d[:])

            wty = cf32[:, _WTY : _WTY + 2 * W]
            wtx = cf32[:, _WTX : _WTX + 2 * W]
            psi2y = cf32[:, _PSIY : _PSIY + R]
            psi2x = cf32[:, _PSIX : _PSIX + R]
            phicat = cf32[0:R, _PHI : _PHI + 2 * CO]
            ysel = cf32[:, _YSEL : _YSEL + 128]
            xrep2 = cf32[:, _XREP2 : _XREP2 + W]
            id1 = cf32[0:1, _ID1 : _ID1 + 1]

            # per-batch reduction vectors, one column per batch
            gy_sb = consts.tile([2 * CI, BPC], F32)
            gx_sb = consts.tile([2 * CI, BPC], F32)

            def stage_a(b, interleave=None):
                """Reduce v[b] -> gy_sb/gx_sb[:, b]."""
                inter = interleave() if interleave is not None else None
                ps = psum_a.tile([128, 2, W], F32, tag="A")
                for blk in range(NIB):
                    t = in_pool.tile([128, IBLK, 2, W], BF16, tag="in")
                    nc.sync.dma_start(
                        t[:],
                        v5[b, :, blk * IBLK : (blk + 1) * IBLK, :, :],
                    )
                    for ii in range(IBLK):
                        ch = blk * IBLK + ii
                        lo = _YLHS + 128 - 2 * ch
                        nc.tensor.matmul(
                            ps[:],
                            lhsT=cf16[:, lo : lo + 128],
                            rhs=t[:, ii, :, :],
                            start=(ch == 0),
                            stop=(ch == CI - 1),
                        )
                    if inter is not None:
                        next(inter, None)
                s2 = scratch.tile([128, 2 * W], F32, tag="s2")
                nc.vector.tensor_copy(s2[:], ps[:].rearrange("p hh w -> p (hh w)"))
                sc = scratch.tile([128, 2 * W], F32, tag="sc")
                nc.vector.tensor_tensor(out=sc[:], in0=s2[:], in1=wty, op=MULT)
                nc.vector.tensor_reduce(
                    out=gy_sb[:, b : b + 1], in_=sc[:],
                    axis=mybir.AxisListType.X, op=ADD,
                )
                sc2 = scratch.tile([128, 2 * W], F32, tag="sc")
                nc.vector.tensor_tensor(out=sc2[:], in0=s2[:], in1=wtx, op=MULT)
                nc.vector.tensor_reduce(
                    out=gx_sb[:, b : b + 1], in_=sc2[:],
                    axis=mybir.AxisListType.X, op=ADD,
                )

            def tiny_gen(b, out):
                """gy/gx[:, b] -> yA/xB gen bases (emitted in 3 pumps)."""
                # inner, transposed directly via contraction-side lhsT
                innert_ps = psum_t.tile([R, 1], F32, tag="tiny")
                nc.tensor.matmul(
                    innert_ps[:], lhsT=psi2y, rhs=gy_sb[:, b : b + 1],
                    start=True, stop=False,
                )
                nc.tensor.matmul(
                    innert_ps[:], lhsT=psi2x, rhs=gx_sb[:, b : b + 1],
                    start=False, stop=True,
                )
                sb_innert = scratch.tile([R, 1], F32, tag="ti2")
                nc.vector.tensor_copy(sb_innert[:], innert_ps[:])
                yield

                ab_ps = psum_t.tile([1, 2 * CO], F32, tag="tiny")
                nc.tensor.matmul(
                    ab_ps[:], lhsT=sb_innert[:], rhs=phicat,
                    start=True, stop=True,
                )
                sb_ab = scratch.tile([1, 2 * CO], F32, tag="ti3")
                nc.vector.tensor_copy(sb_ab[:], ab_ps[:])

                # s_raw[o] = |A[o]| + |B[o]|; ab2 = 127 * ab / s_raw;
                # sdeq[b, o] = s_raw / 127 for host dequantization
                negab = scratch.tile([1, 2 * CO], F32, tag="ti4b")
                nc.vector.tensor_scalar(
                    out=negab[:], in0=sb_ab[:], scalar1=-1.0,
                    scalar2=None, op0=MULT,
                )
                absab = scratch.tile([1, 2 * CO], F32, tag="ti4")
                nc.vector.tensor_tensor(
                    out=absab[:], in0=sb_ab[:], in1=negab[:], op=MAX
                )
                a2 = absab[0:1, :].rearrange("r (o c) -> r o c", c=2)
                sraw = scratch.tile([1, CO], F32, tag="ti5")
                nc.vector.tensor_tensor(
                    out=sraw[:], in0=a2[:, :, 0], in1=a2[:, :, 1], op=ADD
                )
                rcp = scratch.tile([1, CO], F32, tag="ti6b")
                nc.vector.reciprocal(rcp[:], sraw[:])
                rinv = scratch.tile([1, CO], F32, tag="ti6")
                nc.vector.tensor_scalar(
                    out=rinv[:], in0=rcp[:], scalar1=127.0,
                    scalar2=None, op0=MULT,
                )
                ab2 = scratch.tile([1, 2 * CO], F32, tag="ti7")
                ab2v = ab2[0:1, :].rearrange("r (o c) -> r o c", c=2)
                abv = sb_ab[0:1, :].rearrange("r (o c) -> r o c", c=2)
                nc.vector.tensor_tensor(
                    out=ab2v[:, :, 0], in0=abv[:, :, 0], in1=rinv[:], op=MULT
                )
                nc.vector.tensor_tensor(
                    out=ab2v[:, :, 1], in0=abv[:, :, 1], in1=rinv[:], op=MULT
                )
                sdeq_sb = scratch.tile([1, CO], F32, tag="ti8")
                nc.vector.tensor_scalar(
                    out=sdeq_sb[:], in0=sraw[:], scalar1=1.0 / 127.0,
                    scalar2=None, op0=MULT,
                )
                nc.scalar.dma_start(sdeq[b : b + 1, :], sdeq_sb[:])
                yield

                # rows with A'/B' duplicated per (o, j) pair, then PE
                # transposes to per-partition scalar columns
                arow = scratch.tile([1, 128], F32, tag="ti9")
                arv = arow[0:1, :].rearrange("r (o j) -> r o j", j=2)
                nc.vector.tensor_copy(arv[:, :, 0], ab2v[:, :, 0])
                nc.vector.tensor_copy(arv[:, :, 1], ab2v[:, :, 0])
                brow = scratch.tile([1, 128], F32, tag="ti10")
                brv = brow[0:1, :].rearrange("r (o j) -> r o j", j=2)
                nc.vector.tensor_copy(brv[:, :, 0], ab2v[:, :, 1])
                nc.vector.tensor_copy(brv[:, :, 1], ab2v[:, :, 1])
                acol_ps = psum_g.tile([128, 1], F32, tag="ac")
                nc.tensor.transpose(acol_ps[:], arow[:], id1)
                acol = scratch.tile([128, 1], F32, tag="ti11")
                nc.vector.tensor_copy(acol[:], acol_ps[:])
                bcol_ps = psum_g.tile([128, 1], F32, tag="bc")
                nc.tensor.transpose(bcol_ps[:], brow[:], id1)
                bcol = scratch.tile([128, 1], F32, tag="ti12")
                nc.vector.tensor_copy(bcol[:], bcol_ps[:])

                # yA[p, h'] = A'[o] * y[j*128+h'];  xB[p, w] = B'[o] * x[w]
                ya = gen_pool.tile([128, 128], F32, tag="yat")
                nc.vector.tensor_scalar(
                    out=ya[:], in0=ysel, scalar1=acol[:],
                    scalar2=None, op0=MULT,
                )
                xb = gen_pool.tile([128, W], F32, tag="xbt")
                nc.vector.tensor_scalar(
                    out=xb[:], in0=xrep2, scalar1=bcol[:],
                    scalar2=None, op0=MULT,
                )
                out["ya"], out["xb"] = ya, xb
                yield

            def stage_c_gen(b, bases):
                for eng, h0, h1 in _GEN_UNITS:
                    yield
                    n = h1 - h0
                    ya, xb = bases["ya"], bases["xb"]
                    ot = out_pool.tile([128, n, W], I8, tag=f"o{n}")
                    if eng == "act":
                        for hh in range(h0, h1):
                            nc.scalar.activation(
                                ot[:, hh - h0, :], xb[:],
                                mybir.ActivationFunctionType.Identity,
                                bias=ya[:, hh : hh + 1],
                            )
                    elif eng == "pool":
                        for hh in range(h0, h1):
                            nc.gpsimd.tensor_scalar(
                                out=ot[:, hh - h0, :], in0=xb[:],
                                scalar1=1.0, scalar2=ya[:, hh : hh + 1],
                                op0=MULT, op1=ADD,
                            )
                    else:
                        in0 = ya[:, h0:h1].rearrange(
                            "p (h one) -> p h one", one=1
                        )
                        in1 = xb[:].rearrange("p (one w) -> p one w", one=1)
                        i0, i1 = broadcast_tensor_aps(in0, in1)
                        e = nc.vector if eng == "dve" else nc.gpsimd
                        e.tensor_tensor(out=ot[:], in0=i0, in1=i1, op=ADD)
                    nc.scalar.dma_start(u6[b, :, h0:h1, :], ot[:])

            stage_a(0)

            state = {}

            def inter0():
                # Pumped once per A1 chunk: the tiny chain lands in 3 pieces
                # (each PE hop's DVE-side input was produced during the
                # previous chunk's matmuls, so the PE queue never stalls),
                # then gen-0 units flow while batch 1 streams in.
                bases = {}
                state["bases"] = bases
                tg = tiny_gen(0, bases)
                yield from tg
                state["g"] = stage_c_gen(0, bases)
                next(state["g"], None)
                next(state["g"], None)  # unit 0
                yield
                next(state["g"], None)  # unit 1
                yield

            stage_a(1, interleave=inter0)
            bases1 = {}
            for _ in tiny_gen(1, bases1):
                pass
            for _ in state["g"]:        # gen-0 units 2-5
                pass
            for _ in stage_c_gen(1, bases1):
                pass

    nc.compile()
    return nc


def make_in_maps(v, psi, phi):
    import ml_dtypes
    bf16 = ml_dtypes.bfloat16
    y = np.linspace(-1.0, 1.0, H, dtype=np.float32)
    x = np.linspace(-1.0, 1.0, W, dtype=np.float32)
    dx = np.float32(2.0 / (W - 1))
    dy = np.float32(2.0 / (H - 1))

    cf32 = np.zeros((128, CF32_W), np.float32)
    # wty: row 2i = 1 (y_even-weighted sums); row 2i+1 cols [W:2W) = dy
    cf32[0::2, _WTY : _WTY + 2 * W] = 1.0
    cf32[1::2, _WTY + W : _WTY + 2 * W] = dy
    # wtx: row 2i+1 = x (both hh halves); row 1 cols [512:768) doubles as
    # the xB outer-product rhs
    cf32[1::2, _WTX : _WTX + W] = x
    cf32[1::2, _WTX + W : _WTX + 2 * W] = x
    # psi packs (dx folded in)
    cf32[0::2, _PSIY : _PSIY + R] = psi[:, :, 0].T * dx
    cf32[1::2, _PSIY : _PSIY + R] = psi[:, :, 0].T * dx
    cf32[1::2, _PSIX : _PSIX + R] = psi[:, :, 1].T * dx
    # phicat[r, 2o+c] = phi[o, r, c]
    cf32[0:R, _PHI : _PHI + 2 * CO] = np.stack(
        [phi[:, :, 0].T, phi[:, :, 1].T], axis=2
    ).reshape(R, 2 * CO)
    cf32[0::2, _YSEL : _YSEL + 128] = y[:128]
    cf32[1::2, _YSEL : _YSEL + 128] = y[128:]
    cf32[:, _XREP2 : _XREP2 + W] = x
    cf32[0, _ID1] = 1.0

    cf16 = np.zeros((128, CBF16_W), np.float32)
    cf16[:, _YLHS + 128] = y[0::2]
    cf16[:, _YLHS + 129] = 1.0
    cf16 = cf16.astype(bf16)

    # v[b, i, h, w] -> [b, p, i, hh, w] bf16
    v16 = v.astype(bf16)
    vt = v16.reshape(B, CI, HP, 2, W).transpose(0, 2, 1, 3, 4)

    common = {"cf32": cf32, "cf16": cf16}
    return [
        {
            "v5": np.ascontiguousarray(vt[BPC * c : BPC * (c + 1)]),
            **common,
        }
        for c in range(N_CORES)
    ]


def gather_out(results):
    """u6 [BPC, 2o+j, h', w] int8 + sdeq [BPC, CO] -> u [B, CO, H, W] f32."""
    arr = np.stack([r["u6"] for r in results])   # [8, BPC, 128, HQ, W]
    sd = np.stack([r["sdeq"] for r in results])  # [8, BPC, CO]
    arr = arr.reshape(N_CORES, BPC, CO, 2, HQ, W)  # p -> (o, j), in order
    u = arr.astype(np.float32) * sd[:, :, :, None, None, None]
    return np.ascontiguousarray(u.reshape(B, CO, H, W))


_NC_CACHE = None


def kernel(v, psi, phi):
    global _NC_CACHE
    if _NC_CACHE is None:
        _NC_CACHE = build_nc()
    nc = _NC_CACHE
    in_maps = make_in_maps(
        np.asarray(v, dtype=np.float32),
        np.asarray(psi, dtype=np.float32),
        np.asarray(phi, dtype=np.float32),
    )
    res = run_bass_kernel_spmd(nc, in_maps, core_ids=list(range(N_CORES)))
    return gather_out(res.results)


if __name__ == "__main__":
    build_nc()
    print("build ok")


# revision 23
# speedup vs baseline: 1.0672x; 1.0392x over previous
"""Trainium2 Bass kernel for the low-rank linear operator.

Math: the reference collapses algebraically. With y = linspace(-1,1,H),
x = linspace(-1,1,W), dx = 2/(W-1):

  Vy[b,i] = sum_{h,w} v[b,i,h,w] * y_h
  Vx[b,i] = sum_{h,w} v[b,i,h,w] * x_w
  inner[b,r] = dx * sum_i (Vy[b,i]*psi[r,i,0] + Vx[b,i]*psi[r,i,1])
  A[b,o] = sum_r inner[b,r]*phi[o,r,0];  Bc[b,o] = sum_r inner[b,r]*phi[o,r,1]
  u[b,o,h,w] = A[b,o]*y_h + Bc[b,o]*x_w

Sharding: data-parallel over batch, 2 batches per core, 8 cores, no
collectives.

The problem is HBM-bandwidth bound on the way in and generation-engine
bound on the way out. Input streams as bf16 (halves read traffic; rel
err vs the 2e-2 gate measures ~3e-3). Output is written as int8 with a
per-(b,o) dequant scale s = (|A|+|B|)/127 (max|u| = |A|+|B|, so no
clipping); the host multiplies back. int8 quarters write traffic vs f32.

Input reduction: host pre-transposes v to [b, p=h//2, i, hh, w] bf16
(16KB-contiguous per-partition DMA descriptors). For each channel ch one
matmul with a sliding-window lhsT (zeros except col 2ch -> y_even, col
2ch+1 -> ones) over rhs [128, (hh,w)=512] accumulates, for ALL 64
channels, y_even-weighted row sums (psum row 2ch) and column sums (row
2ch+1) into one [128, 512] f32 psum bank. Full-width DVE mult+reduce
passes against wty/wtx give gy/gx; tiny f32 matmuls (contraction-side
lhsT, no transposes) produce inner -> (A,B) -> quantized A' = 127*A/s,
B' = 127*B/s.

Output generation uses an o-on-partitions layout u6[b, p=2o+j, h', w]
(h = j*128 + h') so per-o scalars become per-partition: two tiny PE
outer products build yA[p,h'] = A'[o]*y[j*128+h'] and xB[p,w] =
B'[o]*x[w], then u = yA + xB is generated by WIDE broadcast
tensor_tensor ops (DVE + Pool, ~7K elems/lane per instruction) plus
per-row ACT activations (bias=yA column) -- ~3x fewer engine-cycles of
overhead than per-(o,hh) tensor_scalar tiles. The host reassembly is a
pure reshape (p splits as (o,j) in order) times the dequant scale.

The tiny chain and gen units of batch 0 are emitted interleaved into
batch 1's chunk loop so the PE queue never stalls and the generation
engines stay fed while batch 1 streams in.
"""

import sys

try:
    import concourse.bass as bass  # noqa: F401
except ImportError:
    for _p in ("/opt/trn_rl_repo", "/root/.axon_site/_ro/trn_rl_repo"):
        if _p not in sys.path:
            sys.path.insert(0, _p)

import numpy as np

import concourse.bacc as bacc
import concourse.bass as bass
import concourse.mybir as mybir
import concourse.tile as tile
from concourse.bass import broadcast_tensor_aps
from concourse.bass_utils import run_bass_kernel_spmd

F32 = mybir.dt.float32
BF16 = mybir.dt.bfloat16
I8 = mybir.dt.int8
MULT = mybir.AluOpType.mult
ADD = mybir.AluOpType.add
MAX = mybir.AluOpType.max

B, CI, CO, R, H, W = 16, 64, 64, 64, 256, 256
N_CORES = 8
BPC = B // N_CORES  # batches per core
HP = H // 2         # h-pairs per partition dim
HQ = H // 2         # h' extent in the output layout (h = j*128 + h')

IBLK = 16           # input channels per DMA (2MB bf16, 16KB descriptors)
NIB = CI // IBLK

# gen units: (engine, h'0, h'1) -- DVE gets wide broadcast-TT spans (f32
# in -> int8 out TT is DVE-only); ACT gets activation rows (bias=yA col);
# Pool gets tensor_scalar rows. Shares follow measured engine rates.
_GEN_UNITS = (
    ("dve", 0, 22),
    ("act", 104, 116),
    ("pool", 85, 95),
    ("dve", 22, 43),
    ("act", 116, 128),
    ("pool", 95, 104),
    ("dve", 43, 64),
    ("dve", 64, 85),
)

# packed-constant column offsets (cf32 [128, CF32_W] f32)
_WTY = 0            # [128, 512]
_WTX = 512          # [128, 512]; row 1 cols [512:768) is also x (rhs for xB)
_PSIY = 1024        # [128, 64]
_PSIX = 1088        # [128, 64]
_PHI = 1152         # [64, 128]
_YSEL = 1280        # [128, 128]: Ysel[p, h'] = y[(p%2)*128 + h']
_XREP2 = 1408       # [128, 256]: x replicated on all partitions
_ID1 = 1664         # [1, 1]
CF32_W = 1665
# cf16 [128, CBF16_W] bf16: sliding-window lhsT table
_YLHS = 0           # [128, 384]: col 128 = y_even, col 129 = ones
CBF16_W = 384


def build_nc():
    nc = bacc.Bacc("TRN2", target_bir_lowering=False, debug=False)

    v5 = nc.dram_tensor("v5", [BPC, HP, CI, 2, W], BF16, kind="ExternalInput")
    cf32d = nc.dram_tensor("cf32", [128, CF32_W], F32, kind="ExternalInput")
    cf16d = nc.dram_tensor("cf16", [128, CBF16_W], BF16, kind="ExternalInput")
    u6 = nc.dram_tensor("u6", [BPC, 128, HQ, W], I8, kind="ExternalOutput")
    sdeq = nc.dram_tensor("sdeq", [BPC, CO], F32, kind="ExternalOutput")

    with tile.TileContext(nc) as tc:
        with (
            tc.tile_pool(name="consts", bufs=1) as consts,
            tc.tile_pool(name="inp", bufs=4) as in_pool,
            tc.tile_pool(name="outp", bufs=5) as out_pool,
            tc.tile_pool(name="scr", bufs=3) as scratch,
            tc.tile_pool(name="gen", bufs=4) as gen_pool,
            tc.tile_pool(name="psumA", bufs=2, space="PSUM") as psum_a,
            tc.tile_pool(name="psumT", bufs=1, space="PSUM") as psum_t,
            tc.tile_pool(name="psumG", bufs=2, space="PSUM") as psum_g,
        ):
            # cf16 gates the first matmul: tiny, lands first on the sync ring
            # ahead of the v reads; cf32 rides the scalar ring.
            cf16 = consts.tile([128, CBF16_W], BF16)
            nc.sync.dma_start(cf16[:], cf16d[:])
            cf32 = consts.tile([128, CF32_W], F32)
            nc.scalar.dma_start(cf32[:], cf32d[:])

            wty = cf32[:, _WTY : _WTY + 2 * W]
            wtx = cf32[:, _WTX : _WTX + 2 * W]
            psi2y = cf32[:, _PSIY : _PSIY + R]
            psi2x = cf32[:, _PSIX : _PSIX + R]
            phicat = cf32[0:R, _PHI : _PHI + 2 * CO]
            ysel = cf32[:, _YSEL : _YSEL + 128]
            xrep2 = cf32[:, _XREP2 : _XREP2 + W]
            id1 = cf32[0:1, _ID1 : _ID1 + 1]

            # per-batch reduction vectors, one column per batch
            gy_sb = consts.tile([2 * CI, BPC], F32)
            gx_sb = consts.tile([2 * CI, BPC], F32)

            def stage_a(b, interleave=None):
                """Reduce v[b] -> gy_sb/gx_sb[:, b]."""
                inter = interleave() if interleave is not None else None
                ps = psum_a.tile([128, 2, W], F32, tag="A")
                for blk in range(NIB):
                    t = in_pool.tile([128, IBLK, 2, W], BF16, tag="in")
                    nc.sync.dma_start(
                        t[:],
                        v5[b, :, blk * IBLK : (blk + 1) * IBLK, :, :],
                    )
                    for ii in range(IBLK):
                        ch = blk * IBLK + ii
                        lo = _YLHS + 128 - 2 * ch
                        nc.tensor.matmul(
                            ps[:],
                            lhsT=cf16[:, lo : lo + 128],
                            rhs=t[:, ii, :, :],
                            start=(ch == 0),
                            stop=(ch == CI - 1),
                        )
                    if inter is not None:
                        next(inter, None)
                s2 = scratch.tile([128, 2 * W], F32, tag="s2")
                nc.vector.tensor_copy(s2[:], ps[:].rearrange("p hh w -> p (hh w)"))
                sc = scratch.tile([128, 2 * W], F32, tag="sc")
                nc.vector.tensor_tensor(out=sc[:], in0=s2[:], in1=wty, op=MULT)
                nc.vector.tensor_reduce(
                    out=gy_sb[:, b : b + 1], in_=sc[:],
                    axis=mybir.AxisListType.X, op=ADD,
                )
                sc2 = scratch.tile([128, 2 * W], F32, tag="sc")
                nc.vector.tensor_tensor(out=sc2[:], in0=s2[:], in1=wtx, op=MULT)
                nc.vector.tensor_reduce(
                    out=gx_sb[:, b : b + 1], in_=sc2[:],
                    axis=mybir.AxisListType.X, op=ADD,
                )

            def tiny_gen(b, out):
                """gy/gx[:, b] -> yA/xB gen bases (emitted in 3 pumps)."""
                # inner, transposed directly via contraction-side lhsT
                innert_ps = psum_t.tile([R, 1], F32, tag="tiny")
                nc.tensor.matmul(
                    innert_ps[:], lhsT=psi2y, rhs=gy_sb[:, b : b + 1],
                    start=True, stop=False,
                )
                nc.tensor.matmul(
                    innert_ps[:], lhsT=psi2x, rhs=gx_sb[:, b : b + 1],
                    start=False, stop=True,
                )
                sb_innert = scratch.tile([R, 1], F32, tag="ti2")
                nc.vector.tensor_copy(sb_innert[:], innert_ps[:])
                yield

                ab_ps = psum_t.tile([1, 2 * CO], F32, tag="tiny")
                nc.tensor.matmul(
                    ab_ps[:], lhsT=sb_innert[:], rhs=phicat,
                    start=True, stop=True,
                )
                sb_ab = scratch.tile([1, 2 * CO], F32, tag="ti3")
                nc.vector.tensor_copy(sb_ab[:], ab_ps[:])

                # s_raw[o] = |A[o]| + |B[o]|; ab2 = 127 * ab / s_raw;
                # sdeq[b, o] = s_raw / 127 for host dequantization
                negab = scratch.tile([1, 2 * CO], F32, tag="ti4b")
                nc.vector.tensor_scalar(
                    out=negab[:], in0=sb_ab[:], scalar1=-1.0,
                    scalar2=None, op0=MULT,
                )
                absab = scratch.tile([1, 2 * CO], F32, tag="ti4")
                nc.vector.tensor_tensor(
                    out=absab[:], in0=sb_ab[:], in1=negab[:], op=MAX
                )
                a2 = absab[0:1, :].rearrange("r (o c) -> r o c", c=2)
                sraw = scratch.tile([1, CO], F32, tag="ti5")
                nc.vector.tensor_tensor(
                    out=sraw[:], in0=a2[:, :, 0], in1=a2[:, :, 1], op=ADD
                )
                rcp = scratch.tile([1, CO], F32, tag="ti6b")
                nc.vector.reciprocal(rcp[:], sraw[:])
                rinv = scratch.tile([1, CO], F32, tag="ti6")
                nc.vector.tensor_scalar(
                    out=rinv[:], in0=rcp[:], scalar1=127.0,
                    scalar2=None, op0=MULT,
                )
                ab2 = scratch.tile([1, 2 * CO], F32, tag="ti7")
                ab2v = ab2[0:1, :].rearrange("r (o c) -> r o c", c=2)
                abv = sb_ab[0:1, :].rearrange("r (o c) -> r o c", c=2)
                nc.vector.tensor_tensor(
                    out=ab2v[:, :, 0], in0=abv[:, :, 0], in1=rinv[:], op=MULT
                )
                nc.vector.tensor_tensor(
                    out=ab2v[:, :, 1], in0=abv[:, :, 1], in1=rinv[:], op=MULT
                )
                sdeq_sb = scratch.tile([1, CO], F32, tag="ti8")
                nc.vector.tensor_scalar(
                    out=sdeq_sb[:], in0=sraw[:], scalar1=1.0 / 127.0,
                    scalar2=None, op0=MULT,
                )
                nc.scalar.dma_start(sdeq[b : b + 1, :], sdeq_sb[:])
                yield

                # rows with A'/B' duplicated per (o, j) pair, then PE
                # transposes to per-partition scalar columns
                arow = scratch.tile([1, 128], F32, tag="ti9")
                arv = arow[0:1, :].rearrange("r (o j) -> r o j", j=2)
                nc.vector.tensor_copy(arv[:, :, 0], ab2v[:, :, 0])
                nc.vector.tensor_copy(arv[:, :, 1], ab2v[:, :, 0])
                brow = scratch.tile([1, 128], F32, tag="ti10")
                brv = brow[0:1, :].rearrange("r (o j) -> r o j", j=2)
                nc.vector.tensor_copy(brv[:, :, 0], ab2v[:, :, 1])
                nc.vector.tensor_copy(brv[:, :, 1], ab2v[:, :, 1])
                acol_ps = psum_g.tile([128, 1], F32, tag="ac")
                nc.tensor.transpose(acol_ps[:], arow[:], id1)
                acol = scratch.tile([128, 1], F32, tag="ti11")
                nc.vector.tensor_copy(acol[:], acol_ps[:])
                bcol_ps = psum_g.tile([128, 1], F32, tag="bc")
                nc.tensor.transpose(bcol_ps[:], brow[:], id1)
                bcol = scratch.tile([128, 1], F32, tag="ti12")
                nc.vector.tensor_copy(bcol[:], bcol_ps[:])

                # yA[p, h'] = A'[o] * y[j*128+h'];  xB[p, w] = B'[o] * x[w]
                # bf16 copies feed the wide TTs (2x DVE packing, fast Pool
                # reads); an f32 yA serves as ACT bias / Pool scalar APs.
                ya = gen_pool.tile([128, 128], F32, tag="yat")
                nc.vector.tensor_scalar(
                    out=ya[:], in0=ysel, scalar1=acol[:],
                    scalar2=None, op0=MULT,
                )
                ya16 = gen_pool.tile([128, 128], BF16, tag="yat16")
                nc.vector.tensor_scalar(
                    out=ya16[:], in0=ysel, scalar1=acol[:],
                    scalar2=None, op0=MULT,
                )
                xb16 = gen_pool.tile([128, W], BF16, tag="xbt16")
                nc.vector.tensor_scalar(
                    out=xb16[:], in0=xrep2, scalar1=bcol[:],
                    scalar2=None, op0=MULT,
                )
                out["ya"], out["ya16"], out["xb16"] = ya, ya16, xb16
                yield

            def stage_c_gen(b, bases):
                for eng, h0, h1 in _GEN_UNITS:
                    yield
                    n = h1 - h0
                    ya = bases["ya"]
                    ya16, xb16 = bases["ya16"], bases["xb16"]
                    ot = out_pool.tile([128, n, W], I8, tag=f"o{n}")
                    if eng == "act":
                        for hh in range(h0, h1):
                            nc.scalar.activation(
                                ot[:, hh - h0, :], xb16[:],
                                mybir.ActivationFunctionType.Identity,
                                bias=ya[:, hh : hh + 1],
                            )
                    elif eng == "pool":
                        for hh in range(h0, h1):
                            nc.gpsimd.tensor_scalar(
                                out=ot[:, hh - h0, :], in0=xb16[:],
                                scalar1=1.0, scalar2=ya[:, hh : hh + 1],
                                op0=MULT, op1=ADD,
                            )
                    else:
                        in0 = ya16[:, h0:h1].rearrange(
                            "p (h one) -> p h one", one=1
                        )
                        in1 = xb16[:].rearrange("p (one w) -> p one w", one=1)
                        i0, i1 = broadcast_tensor_aps(in0, in1)
                        nc.vector.tensor_tensor(out=ot[:], in0=i0, in1=i1, op=ADD)
                    nc.scalar.dma_start(u6[b, :, h0:h1, :], ot[:])

            stage_a(0)

            state = {}

            def inter0():
                # Pumped once per A1 chunk: the tiny chain lands in 3 pieces
                # (each PE hop's DVE-side input was produced during the
                # previous chunk's matmuls, so the PE queue never stalls),
                # then gen-0 units flow while batch 1 streams in.
                bases = {}
                state["bases"] = bases
                tg = tiny_gen(0, bases)
                yield from tg
                state["g"] = stage_c_gen(0, bases)
                next(state["g"], None)
                next(state["g"], None)  # unit 0
                yield
                next(state["g"], None)  # unit 1
                yield

            stage_a(1, interleave=inter0)
            bases1 = {}
            for _ in tiny_gen(1, bases1):
                pass
            for _ in state["g"]:        # gen-0 units 2-5
                pass
            for _ in stage_c_gen(1, bases1):
                pass

    nc.compile()
    return nc


def make_in_maps(v, psi, phi):
    import ml_dtypes
    bf16 = ml_dtypes.bfloat16
    y = np.linspace(-1.0, 1.0, H, dtype=np.float32)
    x = np.linspace(-1.0, 1.0, W, dtype=np.float32)
    dx = np.float32(2.0 / (W - 1))
    dy = np.float32(2.0 / (H - 1))

    cf32 = np.zeros((128, CF32_W), np.float32)
    # wty: row 2i = 1 (y_even-weighted sums); row 2i+1 cols [W:2W) = dy
    cf32[0::2, _WTY : _WTY + 2 * W] = 1.0
    cf32[1::2, _WTY + W : _WTY + 2 * W] = dy
    # wtx: row 2i+1 = x (both hh halves); row 1 cols [512:768) doubles as
    # the xB outer-product rhs
    cf32[1::2, _WTX : _WTX + W] = x
    cf32[1::2, _WTX + W : _WTX + 2 * W] = x
    # psi packs (dx folded in)
    cf32[0::2, _PSIY : _PSIY + R] = psi[:, :, 0].T * dx
    cf32[1::2, _PSIY : _PSIY + R] = psi[:, :, 0].T * dx
    cf32[1::2, _PSIX : _PSIX + R] = psi[:, :, 1].T * dx
    # phicat[r, 2o+c] = phi[o, r, c]
    cf32[0:R, _PHI : _PHI + 2 * CO] = np.stack(
        [phi[:, :, 0].T, phi[:, :, 1].T], axis=2
    ).reshape(R, 2 * CO)
    cf32[0::2, _YSEL : _YSEL + 128] = y[:128]
    cf32[1::2, _YSEL : _YSEL + 128] = y[128:]
    cf32[:, _XREP2 : _XREP2 + W] = x
    cf32[0, _ID1] = 1.0

    cf16 = np.zeros((128, CBF16_W), np.float32)
    cf16[:, _YLHS + 128] = y[0::2]
    cf16[:, _YLHS + 129] = 1.0
    cf16 = cf16.astype(bf16)

    # v[b, i, h, w] -> [b, p, i, hh, w] bf16
    v16 = v.astype(bf16)
    vt = v16.reshape(B, CI, HP, 2, W).transpose(0, 2, 1, 3, 4)

    common = {"cf32": cf32, "cf16": cf16}
    return [
        {
            "v5": np.ascontiguousarray(vt[BPC * c : BPC * (c + 1)]),
            **common,
        }
        for c in range(N_CORES)
    ]


def gather_out(results):
    """u6 [BPC, 2o+j, h', w] int8 + sdeq [BPC, CO] -> u [B, CO, H, W] f32."""
    arr = np.stack([r["u6"] for r in results])   # [8, BPC, 128, HQ, W]
    sd = np.stack([r["sdeq"] for r in results])  # [8, BPC, CO]
    arr = arr.reshape(N_CORES, BPC, CO, 2, HQ, W)  # p -> (o, j), in order
    u = arr.astype(np.float32) * sd[:, :, :, None, None, None]
    return np.ascontiguousarray(u.reshape(B, CO, H, W))


_NC_CACHE = None


def kernel(v, psi, phi):
    global _NC_CACHE
    if _NC_CACHE is None:
        _NC_CACHE = build_nc()
    nc = _NC_CACHE
    in_maps = make_in_maps(
        np.asarray(v, dtype=np.float32),
        np.asarray(psi, dtype=np.float32),
        np.asarray(phi, dtype=np.float32),
    )
    res = run_bass_kernel_spmd(nc, in_maps, core_ids=list(range(N_CORES)))
    return gather_out(res.results)


if __name__ == "__main__":
    build_nc()
    print("build ok")
